# revision 22
# baseline (speedup 1.0000x reference)
"""Trainium2 Bass kernel for fake-quant (W8A8) linear: y = fq_tok(x) @ fq_ch(w).T + b.

Full shapes: x [4, 2048, 4096] f32, w [4096, 4096] f32, b [4096] f32.
Sharding over 8 cores: 2 token groups x 4 out-channel groups.
Per core: x_sh [4096, 4096], w_sh [1024, 4096], b_sh [1024] -> y_sh [4096, 1024].

Key ideas:
- fp16 magic rounding: fp16 has 11 significant bits, so converting
  (x*(1/s) + 1536) to fp16 rounds to an EXACT integer + 1536 (all of
  [1409,1663] sit in the [1024,2048) binade where fp16 ulp == 1), in a single
  ACT activation op.  The x side therefore carries u = q + 1536; the shift is
  removed after the matmul using the per-channel weight sum:
      sum_k u_x[t,k]*q_w[c,k] = dot(q_x,q_w) + 1536*sum_k q_w[c,k]
  so  y = sx * (psum*sw - crow) + b,  crow[c] = 1536*sw[c]*sum_k q_w[c,k].
  The w side is kept signed (q_w = u_w - 1536 via a second ACT op; exact in
  fp16) so no per-token sum is needed.  fp16 matmul runs at full PE rate
  (1 cycle/row) with fp32 PSUM accumulation (verified exact on HW).
- All transposes run on the PE (is_transpose matmul through PSUM + ACT copy
  back to SBUF).  The DMA XBAR path was measured and rejected: it moves
  transposed tiles as 256-byte packets at ~21ns each, eating ~40% of the 16
  DMA engines' capacity, and two concurrent XBAR transposes corrupt each
  other (single shared unit).
- Loads are dispatch-queue limited (~130 GB/s effective per HWDGE queue; the
  16 DMA engines sit ~36% busy).  The w tiles are therefore loaded as two
  half-tiles, low half on the sync(SP) queue and high half on the
  scalar(ACT) queue - the scalar-queue dispatches are emitted BEFORE any ACT
  compute so they fire immediately - cutting the w-resident gate roughly in
  half.  x loads stream on the sync queue.
- PE budget: 2048 matmuls (437us) + 1024+256 transposes (69us) ~= 506us at
  2.4 GHz; ACT ~300us; DVE ~310us.  PE is the roofline; matmul blocks are
  cb-outer so channels 0-511 (w row-tiles 0-3) start as soon as those tiles
  are resident.
"""

from contextlib import ExitStack

import numpy as np

import concourse.bass as bass
import concourse.mybir as mybir
import concourse.tile as tile
from concourse import bacc
from concourse.masks import make_identity

P = 128
FP16_MAGIC = 1536.0  # 1.5 * 2**10: [1024, 2048) binade has ulp exactly 1
QMAX = 127.0
EPS = 1e-8

# full problem shapes (hardcoded per harness contract)
B, S, D_IN, D_OUT = 4, 2048, 4096, 4096
TOK = B * S  # 8192
TOK_GROUPS = 2
CH_GROUPS = 4
T_SH = TOK // TOK_GROUPS  # 4096 tokens per core
O_SH = D_OUT // CH_GROUPS  # 1024 channels per core


def build_nc(T, K, O, nch=512):
    """Build the per-core Bass program: x[T,K], w[O,K], b[O] -> y[T,O]."""
    f32 = mybir.dt.float32
    fp16 = mybir.dt.float16
    Copy = mybir.ActivationFunctionType.Copy
    Alu = mybir.AluOpType
    AxX = mybir.AxisListType.X

    assert T % P == 0 and K % P == 0 and O % P == 0
    TT, KB, WT = T // P, K // P, O // P
    NCH = min(nch, O)
    CB = O // NCH
    KH2 = (K // 2)  # w half-tile split point (bytes-contiguous halves)

    nc = bacc.Bacc("TRN2", target_bir_lowering=False, debug=False)
    x_ap = nc.dram_tensor("x", [T, K], f32, kind="ExternalInput").ap()
    w_ap = nc.dram_tensor("w", [O, K], f32, kind="ExternalInput").ap()
    b_ap = nc.dram_tensor("b", [O], f32, kind="ExternalInput").ap()
    y_ap = nc.dram_tensor("y", [T, O], f32, kind="ExternalOutput").ap()

    with tile.TileContext(nc) as tc, ExitStack() as ctx:
        singles = ctx.enter_context(tc.tile_pool(name="singles", bufs=1))
        bigf32 = ctx.enter_context(tc.tile_pool(name="bigf32", bufs=3))
        qpool = ctx.enter_context(tc.tile_pool(name="qpool", bufs=3))
        qtpool = ctx.enter_context(tc.tile_pool(name="qtpool", bufs=4))
        stats = ctx.enter_context(tc.tile_pool(name="stats", bufs=10))
        opool = ctx.enter_context(tc.tile_pool(name="opool", bufs=6))
        psum_pool = ctx.enter_context(tc.tile_pool(name="psum", bufs=4, space="PSUM"))
        tpsum = ctx.enter_context(tc.tile_pool(name="tpsum", bufs=3, space="PSUM"))
        dram = ctx.enter_context(tc.tile_pool(name="dram", bufs=1, space="DRAM"))

        # resident: transposed quantized weights + broadcast rows
        qwT = singles.tile([P, KB, O], fp16)  # qwT[f, k, c] = qw[c, k*128+f]
        sw_b = singles.tile([P, O], f32)  # per-channel scale, bcast rows
        crow_b = singles.tile([P, O], f32)  # 1536*sw*qwsum, bcast rows
        bb_b = singles.tile([P, O], f32)  # bias, bcast rows
        sw_dram = dram.tile([O, 1], f32)
        crow_dram = dram.tile([O, 1], f32)
        ident = singles.tile([P, P], fp16)

        TG = 8  # k-blocks per PE-transpose psum group (8*128 fp16 = one bank)

        # bias broadcast (no deps) + identity
        nc.sync.dma_start(
            out=bb_b,
            in_=bass.AP(tensor=b_ap.tensor, offset=b_ap.offset, ap=[[0, P], [1, O]]),
        )
        make_identity(nc, ident)

        def scale_recip(src_t, s_t, tag):
            # per-row amax -> scale s_t and reciprocal r_t (DVE)
            amax = stats.tile([P, 1], f32, tag="st", name=f"amax_{tag}")
            nc.vector.reduce_max(
                out=amax, in_=src_t, axis=AxX, apply_absolute_value=True
            )
            nc.vector.tensor_scalar(
                out=s_t, in0=amax, scalar1=1.0 / QMAX, scalar2=EPS,
                op0=Alu.mult, op1=Alu.max,
            )
            r_t = stats.tile([P, 1], f32, tag="st", name=f"recip_{tag}")
            nc.vector.reciprocal(out=r_t, in_=s_t)
            return r_t

        def pe_transpose(q_t, dst, tag, dve_copy=False):
            # [128, K] fp16 -> dst[f, k, t] = q_t[t, k*128+f] via PE transpose
            # through tpsum + copy back to SBUF (ACT, or DVE at 2 elem/cycle
            # for fp16 to keep ACT free for the quantize rounds)
            for g in range(KB // TG):
                tp = tpsum.tile([P, TG, P], fp16, tag="tp", name=f"tp_{tag}_{g}")
                for j in range(TG):
                    kb = g * TG + j
                    nc.tensor.transpose(
                        tp[:, j, :], q_t[:, kb * P : (kb + 1) * P], ident
                    )
                if dve_copy:
                    nc.vector.tensor_copy(
                        out=dst[:, g * TG : (g + 1) * TG, :], in_=tp
                    )
                else:
                    nc.scalar.activation(
                        out=dst[:, g * TG : (g + 1) * TG, :], in_=tp, func=Copy
                    )

        def w_chain(wt):
            # load w row-tile (halves split across BOTH HWDGE queues - loads
            # are dispatch-queue limited at ~130 GB/s per queue), quantize to
            # SIGNED fp16 ints (two ACT ops), PE transpose into qwT,
            # per-channel scale + shift-correction rows
            w_t = bigf32.tile([P, K], f32, tag="big", name=f"w_{wt}")
            nc.sync.dma_start(
                out=w_t[:, :KH2], in_=w_ap[wt * P : (wt + 1) * P, :KH2]
            )
            nc.scalar.dma_start(
                out=w_t[:, KH2:], in_=w_ap[wt * P : (wt + 1) * P, KH2:]
            )
            sw = stats.tile([P, 1], f32, tag="st", name=f"sw_{wt}")
            r_t = scale_recip(w_t, sw, f"w{wt}")
            uw = qpool.tile([P, K], fp16, tag="q", name=f"uw_{wt}")
            nc.scalar.activation(
                out=uw, in_=w_t, func=Copy, bias=FP16_MAGIC, scale=r_t[:, 0:1]
            )
            # unshift on DVE (fp16 2x rate) to keep ACT free during the
            # w-phase/x-stream overlap window
            qw = qpool.tile([P, K], fp16, tag="q", name=f"qw_{wt}")
            nc.vector.tensor_scalar_add(out=qw, in0=uw, scalar1=-FP16_MAGIC)
            pe_transpose(qw, qwT[:, :, wt * P : (wt + 1) * P], f"w{wt}")
            # crow = 1536 * sw * sum_k qw   (f32 accumulation on DVE)
            qwsum = stats.tile([P, 1], f32, tag="st", name=f"qwsum_{wt}")
            nc.vector.reduce_sum(out=qwsum, in_=qw, axis=AxX)
            crow = stats.tile([P, 1], f32, tag="st", name=f"crow_{wt}")
            nc.vector.tensor_scalar_mul(out=crow, in0=qwsum, scalar1=FP16_MAGIC)
            crow2 = stats.tile([P, 1], f32, tag="st", name=f"crow2_{wt}")
            nc.vector.tensor_mul(out=crow2, in0=crow, in1=sw)
            nc.sync.dma_start(out=sw_dram[wt * P : (wt + 1) * P, :], in_=sw)
            nc.sync.dma_start(out=crow_dram[wt * P : (wt + 1) * P, :], in_=crow2)

        def x_chain(tt):
            # load x tile, quantize to SHIFTED fp16 ints u = q + 1536 in one
            # ACT op, PE-transpose to qxT
            x_t = bigf32.tile([P, K], f32, tag="big", name=f"x_{tt}")
            nc.sync.dma_start(
                out=x_t[:, :KH2], in_=x_ap[tt * P : (tt + 1) * P, :KH2]
            )
            nc.scalar.dma_start(
                out=x_t[:, KH2:], in_=x_ap[tt * P : (tt + 1) * P, KH2:]
            )
            sx = stats.tile([P, 1], f32, tag="sx", name=f"sx_{tt}")
            r_t = scale_recip(x_t, sx, f"x{tt}")
            ux = qpool.tile([P, K], fp16, tag="q", name=f"ux_{tt}")
            nc.scalar.activation(
                out=ux, in_=x_t, func=Copy, bias=FP16_MAGIC, scale=r_t[:, 0:1]
            )
            qxT = qtpool.tile([P, KB, P], fp16)  # qxT[f, k, t] = ux[t, k*128+f]
            pe_transpose(ux, qxT, f"x{tt}", dve_copy=True)
            return sx, qxT

        def matmul_block(tt, sx, qxT):
            # cb-outer: channel block cb only needs w row-tiles
            # [cb*4, cb*4+4), so cb=0 starts before w tiles 4-7 are resident
            for cb in range(CB):
                psum = psum_pool.tile(
                    [P, NCH], f32, tag="psum", name=f"psum_{tt}_{cb}"
                )
                for k in range(KB):
                    nc.tensor.matmul(
                        psum,
                        qxT[:, k, :],
                        qwT[:, k, cb * NCH : (cb + 1) * NCH],
                        start=(k == 0),
                        stop=(k == KB - 1),
                    )
                # y = sx * (psum*sw - crow) + b
                o1 = opool.tile([P, NCH], f32, tag="o", name=f"o1_{tt}_{cb}")
                nc.vector.tensor_mul(
                    out=o1, in0=psum, in1=sw_b[:, cb * NCH : (cb + 1) * NCH]
                )
                o2 = opool.tile([P, NCH], f32, tag="o", name=f"o2_{tt}_{cb}")
                nc.gpsimd.tensor_sub(
                    out=o2, in0=o1, in1=crow_b[:, cb * NCH : (cb + 1) * NCH]
                )
                o3 = opool.tile([P, NCH], f32, tag="o", name=f"o3_{tt}_{cb}")
                nc.vector.scalar_tensor_tensor(
                    out=o3, in0=o2, scalar=sx[:, 0:1],
                    in1=bb_b[:, cb * NCH : (cb + 1) * NCH],
                    op0=Alu.mult, op1=Alu.add,
                )
                nc.sync.dma_start(
                    out=y_ap[tt * P : (tt + 1) * P, cb * NCH : (cb + 1) * NCH],
                    in_=o3,
                )

        # ---- prologue: 8 w chains with the first 2 x chains interleaved ----
        LOOKAHEAD = 2
        xq = {}  # tt -> (sx, qxT)
        w_chain(0)
        w_chain(1)
        xq[0] = x_chain(0)
        w_chain(2)
        w_chain(3)
        xq[1] = x_chain(1)
        w_chain(4)
        w_chain(5)
        w_chain(6)
        w_chain(7)
        # x2 chain BEFORE the (chain-gated) broadcasts so the sync ring
        # keeps streaming x while the broadcasts wait
        xq[2] = x_chain(2)

        # broadcast per-channel rows (need all 8 w chains' stats)
        nc.sync.dma_start(
            out=sw_b,
            in_=bass.AP(
                tensor=sw_dram.tensor, offset=sw_dram.offset, ap=[[0, P], [1, O]]
            ),
        )
        nc.sync.dma_start(
            out=crow_b,
            in_=bass.AP(
                tensor=crow_dram.tensor, offset=crow_dram.offset, ap=[[0, P], [1, O]]
            ),
        )

        # ---- main loop: 3-tile software pipeline (x0-x2 already emitted) ----
        for tt in range(TT):
            nxt = tt + 3
            if nxt < TT:
                xq[nxt] = x_chain(nxt)
            matmul_block(tt, *xq.pop(tt))
    nc.compile()
    return nc


_cached_nc = None


def _get_nc():
    global _cached_nc
    if _cached_nc is None:
        _cached_nc = build_nc(T_SH, D_IN, O_SH)
    return _cached_nc


def kernel(x: np.ndarray, w: np.ndarray, b: np.ndarray, _trace=False):
    from concourse.bass_utils import run_bass_kernel_spmd

    assert x.shape == (B, S, D_IN) and w.shape == (D_OUT, D_IN) and b.shape == (D_OUT,)
    x2 = np.ascontiguousarray(x.reshape(TOK, D_IN), dtype=np.float32)
    w2 = np.ascontiguousarray(w, dtype=np.float32)
    b2 = np.ascontiguousarray(b, dtype=np.float32)

    in_maps = []
    for core in range(8):
        tg, cg = divmod(core, CH_GROUPS)
        in_maps.append(
            {
                "x": np.ascontiguousarray(x2[tg * T_SH : (tg + 1) * T_SH]),
                "w": np.ascontiguousarray(w2[cg * O_SH : (cg + 1) * O_SH]),
                "b": np.ascontiguousarray(b2[cg * O_SH : (cg + 1) * O_SH]),
            }
        )

    nc = _get_nc()
    res = run_bass_kernel_spmd(nc, in_maps, core_ids=list(range(8)), trace=_trace)

    y = np.empty((TOK, D_OUT), dtype=np.float32)
    for core in range(8):
        tg, cg = divmod(core, CH_GROUPS)
        y[tg * T_SH : (tg + 1) * T_SH, cg * O_SH : (cg + 1) * O_SH] = res.results[
            core
        ]["y"]
    if _trace:
        kernel._last_results = res
    return y.reshape(B, S, D_OUT)


# revision 24
# speedup vs baseline: 1.2287x; 1.2287x over previous
"""Trainium2 Bass kernel for fake-quant (W8A8) linear: y = fq_tok(x) @ fq_ch(w).T + b.

Full shapes: x [4, 2048, 4096] f32, w [4096, 4096] f32, b [4096] f32.
Sharding over 8 cores: 2 token groups x 4 out-channel groups.
Per core: x_sh [4096, 4096], w_sh [1024, 4096], b_sh [1024] -> y_sh [4096, 1024].

Key idea: quantized values are integers in [-127, 127], exactly representable
in bf16, so the matmul runs on the PE array in bf16 (full rate) with fp32 PSUM
accumulation - numerically equivalent to the fp32 reference einsum on the
dequantized values.  Scales are applied in the fp32 epilogue.

Rounding: round-half-to-even via the fp32 magic-constant trick
(v + 1.5*2^23 rounds mantissa to integer; subtract again afterwards),
matching jnp.round.  Clipping to [-128, 127] is a no-op by construction
(|x|/s <= 127 when s = amax/127) so it is skipped.

Engine split: DVE does amax + scale/reciprocal + the fp32 epilogue
(psum*sx*sw, +bias); ACT does the rounding multiply-add, the magic-subtract
(f32->bf16) and the PSUM->SBUF copies of PE-transposed tiles; PE does the
128x128 transposes (is_transpose matmul) + the bf16 matmuls.

v7 over the 658us v1 baseline: loads are dispatch-queue limited at ~130 GB/s
per HWDGE queue (the 16 DMA engines sit ~36% busy; nominal 358 GB/s is not
reachable), and v1's ~120us startup bubble was almost entirely the 16.8MB w
shard crawling in on one queue.  Every 2MB tile load is therefore split into
halves dispatched on BOTH queues (sync + scalar), and the first two x chains
are interleaved into the weight phase so the x pipeline is primed when qwT
completes.  The x pipeline runs chains 2 tiles ahead of matmul blocks so the
PSUM->SBUF copy tail hides under a full matmul block; this needs PSUM slack
(psum 4 + tpsum 3 = 7/8 banks) - at 8/8 banks it faults the device.
"""

from contextlib import ExitStack

import numpy as np

import concourse.bass as bass
import concourse.mybir as mybir
import concourse.tile as tile
from concourse import bacc
from concourse.masks import make_identity

P = 128
MAGIC = 12582912.0  # 1.5 * 2**23
QMAX = 127.0
EPS = 1e-8

# full problem shapes (hardcoded per harness contract)
B, S, D_IN, D_OUT = 4, 2048, 4096, 4096
TOK = B * S  # 8192
TOK_GROUPS = 2
CH_GROUPS = 4
T_SH = TOK // TOK_GROUPS  # 4096 tokens per core
O_SH = D_OUT // CH_GROUPS  # 1024 channels per core


def build_nc(T, K, O, nch=512):
    """Build the per-core Bass program: x[T,K], w[O,K], b[O] -> y[T,O]."""
    f32 = mybir.dt.float32
    bf16 = mybir.dt.bfloat16
    Copy = mybir.ActivationFunctionType.Copy
    Alu = mybir.AluOpType
    AxX = mybir.AxisListType.X

    assert T % P == 0 and K % P == 0 and O % P == 0
    TT, KB, WT = T // P, K // P, O // P
    NCH = min(nch, O)
    CB = O // NCH
    KH2 = K // 2  # load-split point: halves go to the two HWDGE queues

    nc = bacc.Bacc("TRN2", target_bir_lowering=False, debug=False)
    x_ap = nc.dram_tensor("x", [T, K], f32, kind="ExternalInput").ap()
    w_ap = nc.dram_tensor("w", [O, K], f32, kind="ExternalInput").ap()
    b_ap = nc.dram_tensor("b", [O], f32, kind="ExternalInput").ap()
    y_ap = nc.dram_tensor("y", [T, O], f32, kind="ExternalOutput").ap()

    with tile.TileContext(nc) as tc, ExitStack() as ctx:
        singles = ctx.enter_context(tc.tile_pool(name="singles", bufs=1))
        bigf32 = ctx.enter_context(tc.tile_pool(name="bigf32", bufs=3))
        rnd = ctx.enter_context(tc.tile_pool(name="rnd", bufs=2))
        qpool = ctx.enter_context(tc.tile_pool(name="qpool", bufs=2))
        qtpool = ctx.enter_context(tc.tile_pool(name="qtpool", bufs=3))
        stats = ctx.enter_context(tc.tile_pool(name="stats", bufs=12))
        opool = ctx.enter_context(tc.tile_pool(name="opool", bufs=4))
        psum_pool = ctx.enter_context(tc.tile_pool(name="psum", bufs=4, space="PSUM"))
        tpsum = ctx.enter_context(tc.tile_pool(name="tpsum", bufs=3, space="PSUM"))
        dram = ctx.enter_context(tc.tile_pool(name="dram", bufs=1, space="DRAM"))

        # resident: transposed quantized weights + broadcast scale/bias rows
        qwT = singles.tile([P, KB, O], bf16)  # qwT[f, k, c] = qw[c, k*128+f]
        sw_b = singles.tile([P, O], f32)
        bb_b = singles.tile([P, O], f32)
        sw_dram = dram.tile([O, 1], f32)
        ident = singles.tile([P, P], bf16)
        make_identity(nc, ident)

        # bias broadcast has no deps; emit first
        nc.sync.dma_start(
            out=bb_b,
            in_=bass.AP(tensor=b_ap.tensor, offset=b_ap.offset, ap=[[0, P], [1, O]]),
        )

        TG = min(8, KB)  # k-blocks per PE-transpose psum group (8*128 bf16 = one bank)

        def split_load(dst, src_rows):
            # 2MB tile load split across BOTH HWDGE queues (each ~130 GB/s)
            nc.sync.dma_start(out=dst[:, :KH2], in_=src_rows[:, :KH2])
            nc.scalar.dma_start(out=dst[:, KH2:], in_=src_rows[:, KH2:])

        def pe_transpose(q_sbuf, dst, tag):
            # q_sbuf [P, K] bf16 -> dst [P, KB, P] slice view with
            # dst[f, k, c] = q_sbuf[c, k*128+f], via PE transpose + ACT copy
            for g in range(KB // TG):
                tp = tpsum.tile([P, TG, P], bf16, tag="tp", name=f"tp_{tag}_{g}")
                for j in range(TG):
                    kb = g * TG + j
                    nc.tensor.transpose(
                        tp[:, j, :], q_sbuf[:, kb * P : (kb + 1) * P], ident
                    )
                nc.scalar.activation(
                    out=dst[:, g * TG : (g + 1) * TG, :], in_=tp, func=Copy
                )

        def quantize(src_t, q_t, s_t, tag, dve_round=False):
            # per-row amax -> scale (s_t), then round src*(1/s) to q_t (bf16)
            amax = stats.tile([P, 1], f32, tag="st", name=f"amax_{tag}")
            nc.vector.reduce_max(
                out=amax, in_=src_t, axis=AxX, apply_absolute_value=True
            )
            nc.vector.tensor_scalar(
                out=s_t, in0=amax, scalar1=1.0 / QMAX, scalar2=EPS,
                op0=Alu.mult, op1=Alu.max,
            )
            r_t = stats.tile([P, 1], f32, tag="st", name=f"recip_{tag}")
            nc.vector.reciprocal(out=r_t, in_=s_t)
            t_t = rnd.tile([P, K], f32, tag="rnd", name=f"t_{tag}")
            # round on ACT (scale is a per-partition pointer operand; the
            # Bacc event-semaphore pass legalizes its single-wait limit)
            if dve_round:
                # weight phase: DVE does the round so ACT (busy with copies
                # and x rounds during the ramp) is off the critical path
                nc.vector.tensor_scalar(
                    out=t_t, in0=src_t, scalar1=r_t[:, 0:1], scalar2=MAGIC,
                    op0=Alu.mult, op1=Alu.add,
                )
            else:
                nc.scalar.activation(
                    out=t_t, in_=src_t, func=Copy, bias=MAGIC, scale=r_t[:, 0:1]
                )
            nc.scalar.activation(out=q_t, in_=t_t, func=Copy, bias=-MAGIC, scale=1.0)

        def w_chain(wt):
            # weight tile: quantize per-channel (DVE round), transpose to qwT
            w_t = bigf32.tile([P, K], f32, tag="big", name=f"w_{wt}")
            split_load(w_t, w_ap[wt * P : (wt + 1) * P, :])
            sw = stats.tile([P, 1], f32, tag="st", name=f"sw_{wt}")
            qw = qpool.tile([P, K], bf16, tag="q", name=f"qw_{wt}")
            quantize(w_t, qw, sw, f"w{wt}", dve_round=True)
            pe_transpose(qw, qwT[:, :, wt * P : (wt + 1) * P], f"w{wt}")
            nc.sync.dma_start(out=sw_dram[wt * P : (wt + 1) * P, :], in_=sw)

        def x_chain(tt):
            # token tile: quantize per-token (ACT rounds), transpose to qxT
            x_t = bigf32.tile([P, K], f32, tag="big", name=f"x_{tt}")
            split_load(x_t, x_ap[tt * P : (tt + 1) * P, :])
            sx = stats.tile([P, 1], f32, tag="sx", name=f"sx_{tt}")
            qx = qpool.tile([P, K], bf16, tag="q", name=f"qx_{tt}")
            quantize(x_t, qx, sx, f"x{tt}")
            qxT = qtpool.tile([P, KB, P], bf16)  # qxT[f, k, t] = qx[t, k*128+f]
            pe_transpose(qx, qxT, f"x{tt}")
            return sx, qxT

        def matmul_block(tt, sx, qxT):
            psums = [
                psum_pool.tile([P, NCH], f32, tag="psum", name=f"psum_{tt}_{cb}")
                for cb in range(CB)
            ]
            for k in range(KB):
                for cb in range(CB):
                    nc.tensor.matmul(
                        psums[cb],
                        qxT[:, k, :],
                        qwT[:, k, cb * NCH : (cb + 1) * NCH],
                        start=(k == 0),
                        stop=(k == KB - 1),
                    )
            for cb in range(CB):
                o1 = opool.tile([P, NCH], f32, tag="o", name=f"o1_{tt}_{cb}")
                nc.vector.scalar_tensor_tensor(
                    out=o1, in0=psums[cb], scalar=sx[:, 0:1],
                    in1=sw_b[:, cb * NCH : (cb + 1) * NCH],
                    op0=Alu.mult, op1=Alu.mult,
                )
                o2 = opool.tile([P, NCH], f32, tag="o", name=f"o2_{tt}_{cb}")
                nc.vector.tensor_add(
                    out=o2, in0=o1, in1=bb_b[:, cb * NCH : (cb + 1) * NCH]
                )
                nc.sync.dma_start(
                    out=y_ap[tt * P : (tt + 1) * P, cb * NCH : (cb + 1) * NCH], in_=o2
                )

        # ---- weight phase with the first two x chains interleaved so the
        # x pipeline is primed when qwT completes ----
        xq = {}  # tt -> (sx, qxT)
        w_chain(0)
        w_chain(1)
        w_chain(2)
        w_chain(3)
        xq[0] = x_chain(0)
        w_chain(4)
        w_chain(5)
        w_chain(6)
        xq[1] = x_chain(1)
        w_chain(7)

        # broadcast per-channel scale across partitions
        nc.sync.dma_start(
            out=sw_b,
            in_=bass.AP(tensor=sw_dram.tensor, offset=sw_dram.offset, ap=[[0, P], [1, O]]),
        )

        # ---- main loop: chains run 2 tiles ahead of matmul blocks ----
        for tt in range(TT):
            nxt = tt + 2
            if nxt < TT:
                xq[nxt] = x_chain(nxt)
            matmul_block(tt, *xq.pop(tt))
    nc.compile()
    return nc


_cached_nc = None


def _get_nc():
    global _cached_nc
    if _cached_nc is None:
        _cached_nc = build_nc(T_SH, D_IN, O_SH)
    return _cached_nc


def kernel(x: np.ndarray, w: np.ndarray, b: np.ndarray, _trace=False):
    from concourse.bass_utils import run_bass_kernel_spmd

    assert x.shape == (B, S, D_IN) and w.shape == (D_OUT, D_IN) and b.shape == (D_OUT,)
    x2 = np.ascontiguousarray(x.reshape(TOK, D_IN), dtype=np.float32)
    w2 = np.ascontiguousarray(w, dtype=np.float32)
    b2 = np.ascontiguousarray(b, dtype=np.float32)

    in_maps = []
    for core in range(8):
        tg, cg = divmod(core, CH_GROUPS)
        in_maps.append(
            {
                "x": np.ascontiguousarray(x2[tg * T_SH : (tg + 1) * T_SH]),
                "w": np.ascontiguousarray(w2[cg * O_SH : (cg + 1) * O_SH]),
                "b": np.ascontiguousarray(b2[cg * O_SH : (cg + 1) * O_SH]),
            }
        )

    nc = _get_nc()
    res = run_bass_kernel_spmd(nc, in_maps, core_ids=list(range(8)), trace=_trace)

    y = np.empty((TOK, D_OUT), dtype=np.float32)
    for core in range(8):
        tg, cg = divmod(core, CH_GROUPS)
        y[tg * T_SH : (tg + 1) * T_SH, cg * O_SH : (cg + 1) * O_SH] = res.results[
            core
        ]["y"]
    if _trace:
        kernel._last_results = res
    return y.reshape(B, S, D_OUT)
